# revision 11
# baseline (speedup 1.0000x reference)
"""Trainium2 kernel for nn_AttentionGate (topk_masking).

Computation (matching the jax reference):
  scores[b, c] = sum over valid rows r of attn[b, r, c]   (pad == 0 -> all rows valid)
  x[b]  = sorted indices of the top-k (k = S/4) columns by score
  plus constant outputs stride / batch / y.

Strategy:
  - Pure data parallel: batch b -> NeuronCore b. Each core streams its
    64 MB [4096, 4096] f32 matrix from HBM (memory-bound) and reduces the
    row (partition) dimension on the PE array with a ones-vector stationary
    operand, accumulating in PSUM across 32 row-chunks.
  - Host does the top-k of the 4096 column scores per batch. Because the
    reference's fp32 accumulation order (flat sequential over rows; verified
    bitwise == jax CPU einsum) differs from the device's blocked order,
    columns whose device score is within +-DELTA of the k-th largest are
    recomputed on the host with the exact reference accumulation order, so
    the selected index set matches the reference exactly.
"""

import numpy as np

_B = 8
_S = 4096
_P = 128          # SBUF partitions
_NBANK = 512      # fp32 matmul max free dim (one PSUM bank)
_NCORES = 8
_THROUGHPUT = 0.25
_DELTA = 0.05     # boundary window half-width; >> max |device - reference| (~8e-3)

_cached_nc = None


def _build_nc():
    import concourse.bass as bass
    import concourse.mybir as mybir
    from concourse import tile

    nc = bass.Bass()
    attn_in = nc.dram_tensor("attn", [_S, _S], mybir.dt.float32, kind="ExternalInput")
    scores_out = nc.dram_tensor(
        "scores", [1, _S], mybir.dt.float32, kind="ExternalOutput"
    )

    n_banks = _S // _NBANK       # 8 column groups
    grp = 1                      # row chunks per DMA (2 MB per transfer)
    n_grps = _S // (_P * grp)    # 8 group DMAs

    with tile.TileContext(nc) as tc:
        with (
            tc.tile_pool(name="io", bufs=8) as io_pool,
            tc.tile_pool(name="consts", bufs=1) as cpool,
            tc.tile_pool(name="ps", bufs=1, space="PSUM") as ps_pool,
            tc.tile_pool(name="res", bufs=1) as rpool,
        ):
            ones_t = cpool.tile([_P, 1], mybir.dt.float32)
            nc.gpsimd.memset(ones_t[:], 1.0)
            zeros_t = cpool.tile([_P, 1], mybir.dt.float32)
            nc.gpsimd.memset(zeros_t[:], 0.0)

            accs = [
                ps_pool.tile([1, _NBANK], mybir.dt.float32, name=f"acc{j}", tag=f"acc{j}")
                for j in range(n_banks)
            ]

            # Dummy matmul: absorbs the cross-engine dependency on the memset
            # constants into a PE instruction with a single sem wait, so every
            # real matmul below carries only its chunk-DMA wait (the bundled
            # LDWEIGHTS has one HW sync-wait slot). Its output is overwritten
            # by the start=True matmul of chunk 0.
            nc.tensor.matmul(
                accs[0][0:1, 0:1], ones_t[:], zeros_t[:],
                start=True, stop=True, skip_group_check=True,
            )

            for g in range(n_grps):
                chunk = io_pool.tile([_P, grp, _S], mybir.dt.float32)
                src = attn_in[_P * grp * g : _P * grp * (g + 1), :].rearrange(
                    "(c p) x -> p c x", c=grp, p=_P
                )
                # Alternate the two HWDGE rings (SP / ACT) so descriptor
                # generation and completion handling of consecutive DMAs
                # overlap and the SDMA engines stay fed.
                dma_eng = nc.sync if g % 2 == 0 else nc.scalar
                dma_eng.dma_start(chunk[:], src)
                for c in range(grp):
                    for j in range(n_banks):
                        nc.tensor.matmul(
                            accs[j][:],
                            ones_t[:],
                            chunk[:, c, _NBANK * j : _NBANK * (j + 1)],
                            start=(g == 0 and c == 0),
                            stop=(g == n_grps - 1 and c == grp - 1),
                        )

            res = rpool.tile([1, _S], mybir.dt.float32)
            for j in range(n_banks):
                nc.scalar.copy(res[0:1, _NBANK * j : _NBANK * (j + 1)], accs[j][:])
            nc.sync.dma_start(scores_out[:], res[:])

    # Walrus's direct2d DMA lowering accepts a single sync wait per DMACopy,
    # but Tile emits both the PE WAR wait (slot's readers done) and the
    # same-lane DMA WAW wait (slot's previous writer done) on buffer-reuse
    # DMAs. The DMA wait is transitively implied here: the matmuls that
    # incremented the PE semaphore each waited on that very DMA-lane value
    # before reading the slot. Drop it.
    insts = [i for blk in nc.m.functions[0].blocks for i in blk.instructions]
    last_dma = [i for i in insts if type(i).__name__ == "InstDMACopy"][-1]
    out_lane = last_dma.sync_info.on_update[0].ant_name

    for inst in insts:
        ty = type(inst).__name__
        si = inst.sync_info
        if si is None or len(si.on_wait) <= 1:
            continue
        if ty == "InstDMACopy":
            eng = [w for w in si.on_wait if not w.ant_name.startswith("DMAHW")]
            dma = [w for w in si.on_wait if w.ant_name.startswith("DMAHW")]
            assert len(eng) == 1 and len(eng) + len(dma) == len(si.on_wait), (
                f"unexpected DMA wait pattern: {si}"
            )
            inst.sync_info = mybir.SyncInfo(on_wait=eng, on_update=list(si.on_update))
        elif ty == "InstDrain":
            # Kernel-tail drain: every wait is transitively implied by the
            # completion of the final (scores) DMA, whose chain covers all
            # matmuls, copies and chunk DMAs. Keep only that lane's wait.
            keep = [w for w in si.on_wait if w.ant_name == out_lane]
            assert len(keep) == 1, f"unexpected drain wait pattern: {si}"
            inst.sync_info = mybir.SyncInfo(on_wait=keep, on_update=list(si.on_update))
    return nc


def _get_nc():
    global _cached_nc
    if _cached_nc is None:
        _cached_nc = _build_nc()
    return _cached_nc


# Set by test harnesses to capture a profile of the device run.
TRACE = False
LAST_EXEC_NS = None


_ldw_patch_done = False


def _enable_walrus_ldw_opt():
    """The 256 accumulating matmuls all reload the same 1-column ones vector;
    walrus's LDW dedup pass (disabled by default in concourse) removes the
    redundant loads so consecutive matmuls pipeline on the PE."""
    global _ldw_patch_done
    if _ldw_patch_done:
        return
    import concourse.bass_utils as bu

    orig = bu.run_command

    def patched(argv, **kw):
        argv = [
            a.replace("--enable-ldw-opt=false", "--enable-ldw-opt=true")
            if isinstance(a, str)
            else a
            for a in argv
        ]
        return orig(argv, **kw)

    bu.run_command = patched
    _ldw_patch_done = True


def _device_scores(attn):
    """attn: [8, 4096, 4096] f32 -> scores [8, 4096] f32 (device column sums)."""
    global LAST_EXEC_NS
    from concourse.bass_utils import run_bass_kernel_spmd

    _enable_walrus_ldw_opt()

    nc = _get_nc()
    in_maps = [{"attn": np.ascontiguousarray(attn[b])} for b in range(_B)]
    res = run_bass_kernel_spmd(nc, in_maps, list(range(_NCORES)), trace=TRACE)
    LAST_EXEC_NS = res.exec_time_ns
    return np.stack([np.asarray(res.results[b]["scores"]).reshape(_S) for b in range(_B)])


def _seq_colsum(mat, cols):
    """Exact reference-order (flat sequential over rows) f32 sums of selected
    columns of one [S, S] matrix. Bitwise-identical to jax CPU einsum."""
    sl = np.ascontiguousarray(mat[:, cols], dtype=np.float32)
    acc = sl[0].copy()
    for r in range(1, sl.shape[0]):
        acc = acc + sl[r]
    return acc


def _topk_sorted_exact(dev_scores, mat, k, delta):
    """Indices (ascending) of the k largest reference-order scores, using
    device scores for the bulk and exact host recomputation near the boundary.

    dev_scores: [S] device column sums; mat: [S, S] the batch matrix.
    """
    S = dev_scores.shape[0]
    v = np.partition(dev_scores, S - k)[S - k]  # k-th largest device score
    certain = np.nonzero(dev_scores > v + delta)[0]
    if certain.size > k:
        return None  # window too narrow for this data; caller widens/falls back
    cand = np.nonzero(
        (dev_scores >= v - delta) & (dev_scores <= v + delta)
    )[0]
    need = k - certain.size
    if need > cand.size:
        return None
    if need == 0:
        chosen = np.empty(0, dtype=np.int64)
    else:
        exact = _seq_colsum(mat, cand)
        # ties -> lower column index first (cand ascending + stable sort),
        # matching jax.lax.top_k
        order = np.argsort(-exact, kind="stable")
        chosen = cand[np.sort(order[:need])]
    return np.sort(np.concatenate([certain, chosen])).astype(np.int32)


def _reference_numpy(pad, attn):
    """Full-fidelity numpy fallback replicating the jax reference bit-exactly
    (flat sequential f32 accumulation). Used only off the fast path."""
    B, S = pad.shape
    k = max(1, int(S * _THROUGHPUT))
    lens = S - pad.sum(axis=1, dtype=np.float32)
    pos = np.arange(S, dtype=pad.dtype)
    valid = pos[None, :] < lens[:, None]
    vm = valid.astype(np.float32)
    acc = attn[:, 0, :] * vm[:, 0:1]
    for r in range(1, S):
        acc = acc + attn[:, r, :] * vm[:, r : r + 1]
    scores = np.where(valid, acc, -np.inf).astype(np.float32)
    x = np.empty((B, k), np.int32)
    for b in range(B):
        order = np.argsort(-scores[b], kind="stable")
        x[b] = np.sort(order[:k])
    return k, x


def kernel(pad, attn):
    pad = np.asarray(pad)
    attn = np.asarray(attn, dtype=np.float32)
    B, S = pad.shape
    k = max(1, int(S * _THROUGHPUT))

    if (B, S) == (_B, _S) and not np.any(pad):
        dev = _device_scores(attn)
        x = np.empty((B, k), np.int32)
        for b in range(B):
            got = None
            for delta in (_DELTA, 4 * _DELTA, 16 * _DELTA):
                got = _topk_sorted_exact(dev[b], attn[b], k, delta)
                if got is not None:
                    break
            if got is None:
                # device scores unusable for this batch: exact host fallback
                exact = _seq_colsum(attn[b], np.arange(S))
                order = np.argsort(-exact, kind="stable")
                got = np.sort(order[:k]).astype(np.int32)
            x[b] = got
    else:
        k, x = _reference_numpy(pad, attn)

    stride = np.int32(k)
    batch = np.repeat(np.arange(B, dtype=np.int32), k)
    y = np.tile(np.arange(k, dtype=np.int32), B)
    return stride, batch, x.reshape(-1), y


# revision 13
# speedup vs baseline: 1.0855x; 1.0855x over previous
"""Trainium2 kernel for nn_AttentionGate (topk_masking).

Computation (matching the jax reference):
  scores[b, c] = sum over valid rows r of attn[b, r, c]   (pad == 0 -> all rows valid)
  x[b]  = sorted indices of the top-k (k = S/4) columns by score
  plus constant outputs stride / batch / y.

Strategy:
  - Pure data parallel: batch b -> NeuronCore b. Each core streams its
    64 MB [4096, 4096] f32 matrix from HBM (memory-bound) and reduces the
    row (partition) dimension on the PE array with a ones-vector stationary
    operand, accumulating in PSUM across 32 row-chunks.
  - Host does the top-k of the 4096 column scores per batch. Because the
    reference's fp32 accumulation order (flat sequential over rows; verified
    bitwise == jax CPU einsum) differs from the device's blocked order,
    columns whose device score is within +-DELTA of the k-th largest are
    recomputed on the host with the exact reference accumulation order, so
    the selected index set matches the reference exactly.
"""

import numpy as np

_B = 8
_S = 4096
_P = 128          # SBUF partitions
_NBANK = 512      # fp32 matmul max free dim (one PSUM bank)
_NCORES = 8
_THROUGHPUT = 0.25
_DELTA = 0.05     # boundary window half-width; >> max |device - reference| (~8e-3)

_cached_nc = None


def _build_nc():
    import concourse.bass as bass
    import concourse.mybir as mybir
    from concourse import tile

    nc = bass.Bass()
    attn_in = nc.dram_tensor("attn", [_S, _S], mybir.dt.float32, kind="ExternalInput")
    scores_out = nc.dram_tensor(
        "scores", [1, _S], mybir.dt.float32, kind="ExternalOutput"
    )

    n_banks = _S // _NBANK       # 8 column groups / PSUM banks
    n_chunks = _S // _P          # 32 row chunks
    CV = 1536                    # columns accumulated on the vector engine
    NB_V = CV // _NBANK          # 3 banks reduced from the DVE accumulator
    NB_PE = n_banks - NB_V       # 5 banks streamed through the PE

    with tile.TileContext(nc) as tc:
        with (
            tc.tile_pool(name="ioa", bufs=8) as pool_a,
            tc.tile_pool(name="iob", bufs=8) as pool_b,
            tc.tile_pool(name="consts", bufs=1) as cpool,
            tc.tile_pool(name="ps", bufs=1, space="PSUM") as ps_pool,
            tc.tile_pool(name="res", bufs=1) as rpool,
        ):
            ones_t = cpool.tile([_P, 1], mybir.dt.float32)
            nc.gpsimd.memset(ones_t[:], 1.0)
            zeros_t = cpool.tile([_P, 1], mybir.dt.float32)
            nc.gpsimd.memset(zeros_t[:], 0.0)
            acc_v = cpool.tile([_P, CV], mybir.dt.float32)

            accs = [
                ps_pool.tile([1, _NBANK], mybir.dt.float32, name=f"acc{j}", tag=f"acc{j}")
                for j in range(n_banks)
            ]

            # Dummy matmul: absorbs the cross-engine dependency on the memset
            # constants into a PE instruction with a single sem wait, so every
            # real matmul below carries only its chunk-DMA wait (the bundled
            # LDWEIGHTS has one HW sync-wait slot). Its output is overwritten
            # by the start=True matmul of chunk 0.
            nc.tensor.matmul(
                accs[NB_V][0:1, 0:1], ones_t[:], zeros_t[:],
                start=True, stop=True, skip_group_check=True,
            )

            for i in range(n_chunks):
                rows = slice(_P * i, _P * (i + 1))
                ta = pool_a.tile([_P, CV], mybir.dt.float32)
                tb = pool_b.tile([_P, _S - CV], mybir.dt.float32)
                nc.sync.dma_start(ta[:], attn_in[rows, 0:CV])
                nc.sync.dma_start(tb[:], attn_in[rows, CV:_S])
                # cols [0, CV): elementwise accumulate across chunks on DVE
                if i == 0:
                    nc.vector.tensor_copy(acc_v[:], ta[:])
                else:
                    nc.vector.tensor_add(acc_v[:], acc_v[:], ta[:])
                # cols [CV, S): partition-reduce on PE, accumulating in PSUM
                for j in range(NB_PE):
                    nc.tensor.matmul(
                        accs[NB_V + j][:],
                        ones_t[:],
                        tb[:, _NBANK * j : _NBANK * (j + 1)],
                        start=(i == 0),
                        stop=(i == n_chunks - 1),
                    )

            # partition-reduce the DVE accumulator into the remaining banks
            for j in range(NB_V):
                nc.tensor.matmul(
                    accs[j][:],
                    ones_t[:],
                    acc_v[:, _NBANK * j : _NBANK * (j + 1)],
                    start=True,
                    stop=True,
                )

            res = rpool.tile([1, _S], mybir.dt.float32)
            for j in range(n_banks):
                nc.scalar.copy(res[0:1, _NBANK * j : _NBANK * (j + 1)], accs[j][:])
            nc.sync.dma_start(scores_out[:], res[:])

    # Walrus's direct2d DMA lowering accepts a single sync wait per DMACopy,
    # but Tile emits both the PE WAR wait (slot's readers done) and the
    # same-lane DMA WAW wait (slot's previous writer done) on buffer-reuse
    # DMAs. The DMA wait is transitively implied here: the matmuls that
    # incremented the PE semaphore each waited on that very DMA-lane value
    # before reading the slot. Drop it.
    insts = [i for blk in nc.m.functions[0].blocks for i in blk.instructions]
    last_dma = [i for i in insts if type(i).__name__ == "InstDMACopy"][-1]
    out_lane = last_dma.sync_info.on_update[0].ant_name

    for inst in insts:
        ty = type(inst).__name__
        si = inst.sync_info
        if si is None or len(si.on_wait) <= 1:
            continue
        if ty == "InstDMACopy":
            eng = [w for w in si.on_wait if not w.ant_name.startswith("DMAHW")]
            dma = [w for w in si.on_wait if w.ant_name.startswith("DMAHW")]
            assert len(eng) == 1 and len(eng) + len(dma) == len(si.on_wait), (
                f"unexpected DMA wait pattern: {si}"
            )
            inst.sync_info = mybir.SyncInfo(on_wait=eng, on_update=list(si.on_update))
        elif ty in ("InstTensorTensor", "InstTensorCopy"):
            # DVE ops: Tile emits a redundant same-engine ordering wait
            # (consecutive DVE ops already serialize in hardware through the
            # per-op pipeline DRAIN) alongside the chunk-DMA data wait.
            keep = [w for w in si.on_wait if not w.ant_name.startswith("DVE")]
            drop = [w for w in si.on_wait if w.ant_name.startswith("DVE")]
            assert len(keep) == 1 and len(drop) == len(si.on_wait) - 1, (
                f"unexpected DVE wait pattern: {si}"
            )
            inst.sync_info = mybir.SyncInfo(on_wait=keep, on_update=list(si.on_update))
        elif ty == "InstDrain":
            # Kernel-tail drain: every wait is transitively implied by the
            # completion of the final (scores) DMA, whose chain covers all
            # matmuls, copies and chunk DMAs. Keep only that lane's wait.
            keep = [w for w in si.on_wait if w.ant_name == out_lane]
            assert len(keep) == 1, f"unexpected drain wait pattern: {si}"
            inst.sync_info = mybir.SyncInfo(on_wait=keep, on_update=list(si.on_update))
    return nc


def _get_nc():
    global _cached_nc
    if _cached_nc is None:
        _cached_nc = _build_nc()
    return _cached_nc


# Set by test harnesses to capture a profile of the device run.
TRACE = False
LAST_EXEC_NS = None


_ldw_patch_done = False


def _enable_walrus_ldw_opt():
    """The 256 accumulating matmuls all reload the same 1-column ones vector;
    walrus's LDW dedup pass (disabled by default in concourse) removes the
    redundant loads so consecutive matmuls pipeline on the PE."""
    global _ldw_patch_done
    if _ldw_patch_done:
        return
    import concourse.bass_utils as bu

    orig = bu.run_command

    def patched(argv, **kw):
        argv = [
            a.replace("--enable-ldw-opt=false", "--enable-ldw-opt=true")
            if isinstance(a, str)
            else a
            for a in argv
        ]
        return orig(argv, **kw)

    bu.run_command = patched
    _ldw_patch_done = True


def _device_scores(attn):
    """attn: [8, 4096, 4096] f32 -> scores [8, 4096] f32 (device column sums)."""
    global LAST_EXEC_NS
    from concourse.bass_utils import run_bass_kernel_spmd

    _enable_walrus_ldw_opt()

    nc = _get_nc()
    in_maps = [{"attn": np.ascontiguousarray(attn[b])} for b in range(_B)]
    res = run_bass_kernel_spmd(nc, in_maps, list(range(_NCORES)), trace=TRACE)
    LAST_EXEC_NS = res.exec_time_ns
    return np.stack([np.asarray(res.results[b]["scores"]).reshape(_S) for b in range(_B)])


def _seq_colsum(mat, cols):
    """Exact reference-order (flat sequential over rows) f32 sums of selected
    columns of one [S, S] matrix. Bitwise-identical to jax CPU einsum."""
    sl = np.ascontiguousarray(mat[:, cols], dtype=np.float32)
    acc = sl[0].copy()
    for r in range(1, sl.shape[0]):
        acc = acc + sl[r]
    return acc


def _topk_sorted_exact(dev_scores, mat, k, delta):
    """Indices (ascending) of the k largest reference-order scores, using
    device scores for the bulk and exact host recomputation near the boundary.

    dev_scores: [S] device column sums; mat: [S, S] the batch matrix.
    """
    S = dev_scores.shape[0]
    v = np.partition(dev_scores, S - k)[S - k]  # k-th largest device score
    certain = np.nonzero(dev_scores > v + delta)[0]
    if certain.size > k:
        return None  # window too narrow for this data; caller widens/falls back
    cand = np.nonzero(
        (dev_scores >= v - delta) & (dev_scores <= v + delta)
    )[0]
    need = k - certain.size
    if need > cand.size:
        return None
    if need == 0:
        chosen = np.empty(0, dtype=np.int64)
    else:
        exact = _seq_colsum(mat, cand)
        # ties -> lower column index first (cand ascending + stable sort),
        # matching jax.lax.top_k
        order = np.argsort(-exact, kind="stable")
        chosen = cand[np.sort(order[:need])]
    return np.sort(np.concatenate([certain, chosen])).astype(np.int32)


def _reference_numpy(pad, attn):
    """Full-fidelity numpy fallback replicating the jax reference bit-exactly
    (flat sequential f32 accumulation). Used only off the fast path."""
    B, S = pad.shape
    k = max(1, int(S * _THROUGHPUT))
    lens = S - pad.sum(axis=1, dtype=np.float32)
    pos = np.arange(S, dtype=pad.dtype)
    valid = pos[None, :] < lens[:, None]
    vm = valid.astype(np.float32)
    acc = attn[:, 0, :] * vm[:, 0:1]
    for r in range(1, S):
        acc = acc + attn[:, r, :] * vm[:, r : r + 1]
    scores = np.where(valid, acc, -np.inf).astype(np.float32)
    x = np.empty((B, k), np.int32)
    for b in range(B):
        order = np.argsort(-scores[b], kind="stable")
        x[b] = np.sort(order[:k])
    return k, x


def kernel(pad, attn):
    pad = np.asarray(pad)
    attn = np.asarray(attn, dtype=np.float32)
    B, S = pad.shape
    k = max(1, int(S * _THROUGHPUT))

    if (B, S) == (_B, _S) and not np.any(pad):
        dev = _device_scores(attn)
        x = np.empty((B, k), np.int32)
        for b in range(B):
            got = None
            for delta in (_DELTA, 4 * _DELTA, 16 * _DELTA):
                got = _topk_sorted_exact(dev[b], attn[b], k, delta)
                if got is not None:
                    break
            if got is None:
                # device scores unusable for this batch: exact host fallback
                exact = _seq_colsum(attn[b], np.arange(S))
                order = np.argsort(-exact, kind="stable")
                got = np.sort(order[:k]).astype(np.int32)
            x[b] = got
    else:
        k, x = _reference_numpy(pad, attn)

    stride = np.int32(k)
    batch = np.repeat(np.arange(B, dtype=np.int32), k)
    y = np.tile(np.arange(k, dtype=np.int32), B)
    return stride, batch, x.reshape(-1), y


# revision 15
# speedup vs baseline: 1.1313x; 1.0422x over previous
"""Trainium2 kernel for nn_AttentionGate (topk_masking).

Computation (matching the jax reference):
  scores[b, c] = sum over valid rows r of attn[b, r, c]   (pad == 0 -> all rows valid)
  x[b]  = sorted indices of the top-k (k = S/4) columns by score
  plus constant outputs stride / batch / y.

Strategy:
  - Pure data parallel: batch b -> NeuronCore b. Each core streams its
    64 MB [4096, 4096] f32 matrix from HBM (memory-bound) and reduces the
    row (partition) dimension on the PE array with a ones-vector stationary
    operand, accumulating in PSUM across 32 row-chunks.
  - Host does the top-k of the 4096 column scores per batch. Because the
    reference's fp32 accumulation order (flat sequential over rows; verified
    bitwise == jax CPU einsum) differs from the device's blocked order,
    columns whose device score is within +-DELTA of the k-th largest are
    recomputed on the host with the exact reference accumulation order, so
    the selected index set matches the reference exactly.
"""

import numpy as np

_B = 8
_S = 4096
_P = 128          # SBUF partitions
_NBANK = 512      # fp32 matmul max free dim (one PSUM bank)
_NCORES = 8
_THROUGHPUT = 0.25
_DELTA = 0.05     # boundary window half-width; >> max |device - reference| (~8e-3)

_cached_nc = None


def _build_nc():
    import concourse.bass as bass
    import concourse.mybir as mybir
    from concourse import tile

    nc = bass.Bass()
    attn_in = nc.dram_tensor("attn", [_S, _S], mybir.dt.float32, kind="ExternalInput")
    scores_out = nc.dram_tensor(
        "scores", [1, _S], mybir.dt.float32, kind="ExternalOutput"
    )

    n_banks = _S // _NBANK       # 8 column groups / PSUM banks
    n_chunks = _S // _P          # 32 row chunks
    CV = 1536                    # columns accumulated on the vector engine
    NB_V = CV // _NBANK          # 3 banks reduced from the DVE accumulator
    NB_PE = n_banks - NB_V       # 5 banks streamed through the PE

    with tile.TileContext(nc) as tc:
        with (
            tc.tile_pool(name="ioa", bufs=8) as pool_a,
            tc.tile_pool(name="iob", bufs=8) as pool_b,
            tc.tile_pool(name="consts", bufs=1) as cpool,
            tc.tile_pool(name="ps", bufs=1, space="PSUM") as ps_pool,
            tc.tile_pool(name="res", bufs=1) as rpool,
        ):
            ones_t = cpool.tile([_P, 1], mybir.dt.float32)
            nc.gpsimd.memset(ones_t[:], 1.0)
            zeros_t = cpool.tile([_P, 1], mybir.dt.float32)
            nc.gpsimd.memset(zeros_t[:], 0.0)
            acc_v = cpool.tile([_P, CV], mybir.dt.float32)

            accs = [
                ps_pool.tile([1, _NBANK], mybir.dt.float32, name=f"acc{j}", tag=f"acc{j}")
                for j in range(n_banks)
            ]

            # Dummy matmul: absorbs the cross-engine dependency on the memset
            # constants into a PE instruction with a single sem wait, so every
            # real matmul below carries only its chunk-DMA wait (the bundled
            # LDWEIGHTS has one HW sync-wait slot). Its output is overwritten
            # by the start=True matmul of chunk 0.
            nc.tensor.matmul(
                accs[NB_V][0:1, 0:1], ones_t[:], zeros_t[:],
                start=True, stop=True, skip_group_check=True,
            )

            for i in range(n_chunks):
                rows = slice(_P * i, _P * (i + 1))
                ta = pool_a.tile([_P, CV], mybir.dt.float32)
                tb = pool_b.tile([_P, _S - CV], mybir.dt.float32)
                nc.sync.dma_start(ta[:], attn_in[rows, 0:CV])
                nc.sync.dma_start(tb[:], attn_in[rows, CV:_S])
                # cols [0, CV): elementwise accumulate across chunks on DVE.
                # The tiny scratch copy takes the chunk-DMA wait onto the DVE
                # instruction stream first, so the accumulate op below carries
                # only Tile's same-engine pipeline-hazard wait (walrus allows
                # a single sync wait per instruction).
                sc = cpool.tile([_P, 1], mybir.dt.float32, name=f"sc{i}", tag=f"sc{i}")
                nc.vector.tensor_copy(sc[:], ta[:, 0:1])
                if i == 0:
                    nc.vector.tensor_copy(acc_v[:], ta[:])
                else:
                    nc.vector.tensor_add(acc_v[:], acc_v[:], ta[:])
                # cols [CV, S): partition-reduce on PE, accumulating in PSUM
                for j in range(NB_PE):
                    nc.tensor.matmul(
                        accs[NB_V + j][:],
                        ones_t[:],
                        tb[:, _NBANK * j : _NBANK * (j + 1)],
                        start=(i == 0),
                        stop=(i == n_chunks - 1),
                    )

            # partition-reduce the DVE accumulator into the remaining banks
            for j in range(NB_V):
                nc.tensor.matmul(
                    accs[j][:],
                    ones_t[:],
                    acc_v[:, _NBANK * j : _NBANK * (j + 1)],
                    start=True,
                    stop=True,
                )

            res = rpool.tile([1, _S], mybir.dt.float32)
            for j in range(n_banks):
                nc.scalar.copy(res[0:1, _NBANK * j : _NBANK * (j + 1)], accs[j][:])
            nc.sync.dma_start(scores_out[:], res[:])

    # Walrus's direct2d DMA lowering accepts a single sync wait per DMACopy,
    # but Tile emits both the PE WAR wait (slot's readers done) and the
    # same-lane DMA WAW wait (slot's previous writer done) on buffer-reuse
    # DMAs. The DMA wait is transitively implied here: the matmuls that
    # incremented the PE semaphore each waited on that very DMA-lane value
    # before reading the slot. Drop it.
    insts = [i for blk in nc.m.functions[0].blocks for i in blk.instructions]
    last_dma = [i for i in insts if type(i).__name__ == "InstDMACopy"][-1]
    out_lane = last_dma.sync_info.on_update[0].ant_name

    for inst in insts:
        ty = type(inst).__name__
        si = inst.sync_info
        if si is None or len(si.on_wait) <= 1:
            continue
        if ty == "InstDMACopy":
            eng = [w for w in si.on_wait if not w.ant_name.startswith("DMAHW")]
            dma = [w for w in si.on_wait if w.ant_name.startswith("DMAHW")]
            assert len(eng) == 1 and len(eng) + len(dma) == len(si.on_wait), (
                f"unexpected DMA wait pattern: {si}"
            )
            inst.sync_info = mybir.SyncInfo(on_wait=eng, on_update=list(si.on_update))
        elif ty == "InstDrain":
            # Kernel-tail drain: every wait is transitively implied by the
            # completion of the final (scores) DMA, whose chain covers all
            # matmuls, copies and chunk DMAs. Keep only that lane's wait.
            keep = [w for w in si.on_wait if w.ant_name == out_lane]
            assert len(keep) == 1, f"unexpected drain wait pattern: {si}"
            inst.sync_info = mybir.SyncInfo(on_wait=keep, on_update=list(si.on_update))
    return nc


def _get_nc():
    global _cached_nc
    if _cached_nc is None:
        _cached_nc = _build_nc()
    return _cached_nc


# Set by test harnesses to capture a profile of the device run.
TRACE = False
LAST_EXEC_NS = None


_ldw_patch_done = False


def _enable_walrus_ldw_opt():
    """The 256 accumulating matmuls all reload the same 1-column ones vector;
    walrus's LDW dedup pass (disabled by default in concourse) removes the
    redundant loads so consecutive matmuls pipeline on the PE."""
    global _ldw_patch_done
    if _ldw_patch_done:
        return
    import concourse.bass_utils as bu

    orig = bu.run_command

    def patched(argv, **kw):
        argv = [
            a.replace("--enable-ldw-opt=false", "--enable-ldw-opt=true")
            if isinstance(a, str)
            else a
            for a in argv
        ]
        return orig(argv, **kw)

    bu.run_command = patched
    _ldw_patch_done = True


def _device_scores(attn):
    """attn: [8, 4096, 4096] f32 -> scores [8, 4096] f32 (device column sums)."""
    global LAST_EXEC_NS
    from concourse.bass_utils import run_bass_kernel_spmd

    _enable_walrus_ldw_opt()

    nc = _get_nc()
    in_maps = [{"attn": np.ascontiguousarray(attn[b])} for b in range(_B)]
    res = run_bass_kernel_spmd(nc, in_maps, list(range(_NCORES)), trace=TRACE)
    LAST_EXEC_NS = res.exec_time_ns
    return np.stack([np.asarray(res.results[b]["scores"]).reshape(_S) for b in range(_B)])


def _seq_colsum(mat, cols):
    """Exact reference-order (flat sequential over rows) f32 sums of selected
    columns of one [S, S] matrix. Bitwise-identical to jax CPU einsum."""
    sl = np.ascontiguousarray(mat[:, cols], dtype=np.float32)
    acc = sl[0].copy()
    for r in range(1, sl.shape[0]):
        acc = acc + sl[r]
    return acc


def _topk_sorted_exact(dev_scores, mat, k, delta):
    """Indices (ascending) of the k largest reference-order scores, using
    device scores for the bulk and exact host recomputation near the boundary.

    dev_scores: [S] device column sums; mat: [S, S] the batch matrix.
    """
    S = dev_scores.shape[0]
    v = np.partition(dev_scores, S - k)[S - k]  # k-th largest device score
    certain = np.nonzero(dev_scores > v + delta)[0]
    if certain.size > k:
        return None  # window too narrow for this data; caller widens/falls back
    cand = np.nonzero(
        (dev_scores >= v - delta) & (dev_scores <= v + delta)
    )[0]
    need = k - certain.size
    if need > cand.size:
        return None
    if need == 0:
        chosen = np.empty(0, dtype=np.int64)
    else:
        exact = _seq_colsum(mat, cand)
        # ties -> lower column index first (cand ascending + stable sort),
        # matching jax.lax.top_k
        order = np.argsort(-exact, kind="stable")
        chosen = cand[np.sort(order[:need])]
    return np.sort(np.concatenate([certain, chosen])).astype(np.int32)


def _reference_numpy(pad, attn):
    """Full-fidelity numpy fallback replicating the jax reference bit-exactly
    (flat sequential f32 accumulation). Used only off the fast path."""
    B, S = pad.shape
    k = max(1, int(S * _THROUGHPUT))
    lens = S - pad.sum(axis=1, dtype=np.float32)
    pos = np.arange(S, dtype=pad.dtype)
    valid = pos[None, :] < lens[:, None]
    vm = valid.astype(np.float32)
    acc = attn[:, 0, :] * vm[:, 0:1]
    for r in range(1, S):
        acc = acc + attn[:, r, :] * vm[:, r : r + 1]
    scores = np.where(valid, acc, -np.inf).astype(np.float32)
    x = np.empty((B, k), np.int32)
    for b in range(B):
        order = np.argsort(-scores[b], kind="stable")
        x[b] = np.sort(order[:k])
    return k, x


def kernel(pad, attn):
    pad = np.asarray(pad)
    attn = np.asarray(attn, dtype=np.float32)
    B, S = pad.shape
    k = max(1, int(S * _THROUGHPUT))

    if (B, S) == (_B, _S) and not np.any(pad):
        dev = _device_scores(attn)
        x = np.empty((B, k), np.int32)
        for b in range(B):
            got = None
            for delta in (_DELTA, 4 * _DELTA, 16 * _DELTA):
                got = _topk_sorted_exact(dev[b], attn[b], k, delta)
                if got is not None:
                    break
            if got is None:
                # device scores unusable for this batch: exact host fallback
                exact = _seq_colsum(attn[b], np.arange(S))
                order = np.argsort(-exact, kind="stable")
                got = np.sort(order[:k]).astype(np.int32)
            x[b] = got
    else:
        k, x = _reference_numpy(pad, attn)

    stride = np.int32(k)
    batch = np.repeat(np.arange(B, dtype=np.int32), k)
    y = np.tile(np.arange(k, dtype=np.int32), B)
    return stride, batch, x.reshape(-1), y


# revision 16
# speedup vs baseline: 1.3357x; 1.1806x over previous
"""Trainium2 kernel for nn_AttentionGate (topk_masking).

Computation (matching the jax reference):
  scores[b, c] = sum over valid rows r of attn[b, r, c]   (pad == 0 -> all rows valid)
  x[b]  = sorted indices of the top-k (k = S/4) columns by score
  plus constant outputs stride / batch / y.

Strategy:
  - Pure data parallel: batch b -> NeuronCore b. Each core streams its
    64 MB [4096, 4096] f32 matrix from HBM (memory-bound) and reduces the
    row (partition) dimension on the PE array with a ones-vector stationary
    operand, accumulating in PSUM across 32 row-chunks.
  - Host does the top-k of the 4096 column scores per batch. Because the
    reference's fp32 accumulation order (flat sequential over rows; verified
    bitwise == jax CPU einsum) differs from the device's blocked order,
    columns whose device score is within +-DELTA of the k-th largest are
    recomputed on the host with the exact reference accumulation order, so
    the selected index set matches the reference exactly.
"""

import numpy as np

_B = 8
_S = 4096
_P = 128          # SBUF partitions
_NBANK = 512      # fp32 matmul max free dim (one PSUM bank)
_NCORES = 8
_THROUGHPUT = 0.25
_DELTA = 0.05     # boundary window half-width; >> max |device - reference| (~8e-3)

_cached_nc = None


def _build_nc():
    import concourse.bass as bass
    import concourse.mybir as mybir
    from concourse import tile

    nc = bass.Bass()
    attn_in = nc.dram_tensor("attn", [_S, _S], mybir.dt.float32, kind="ExternalInput")
    scores_out = nc.dram_tensor(
        "scores", [1, _S], mybir.dt.float32, kind="ExternalOutput"
    )

    n_banks = _S // _NBANK       # 8 column groups / PSUM banks
    n_chunks = _S // _P          # 32 row chunks
    CV = 1536                    # columns accumulated on the vector engine
    NB_V = CV // _NBANK          # 3 banks reduced from the DVE accumulator
    NB_PE = n_banks - NB_V       # 5 banks streamed through the PE

    with tile.TileContext(nc) as tc:
        with (
            tc.tile_pool(name="ioa", bufs=4) as pool_a,
            tc.tile_pool(name="iob", bufs=4) as pool_b,
            tc.tile_pool(name="consts", bufs=1) as cpool,
            tc.tile_pool(name="ps", bufs=1, space="PSUM") as ps_pool,
            tc.tile_pool(name="res", bufs=1) as rpool,
        ):
            ones_t = cpool.tile([_P, 1], mybir.dt.float32)
            nc.gpsimd.memset(ones_t[:], 1.0)
            zeros_t = cpool.tile([_P, 1], mybir.dt.float32)
            nc.gpsimd.memset(zeros_t[:], 0.0)
            acc_v = cpool.tile([_P, CV], mybir.dt.float32)

            accs = [
                ps_pool.tile([1, _NBANK], mybir.dt.float32, name=f"acc{j}", tag=f"acc{j}")
                for j in range(n_banks)
            ]

            # Dummy matmul: absorbs the cross-engine dependency on the memset
            # constants into a PE instruction with a single sem wait, so every
            # real matmul below carries only its chunk-DMA wait (the bundled
            # LDWEIGHTS has one HW sync-wait slot). Its output is overwritten
            # by the start=True matmul of chunk 0.
            nc.tensor.matmul(
                accs[NB_V][0:1, 0:1], ones_t[:], zeros_t[:],
                start=True, stop=True, skip_group_check=True,
            )

            for i in range(n_chunks):
                rows = slice(_P * i, _P * (i + 1))
                ta = pool_a.tile([_P, CV], mybir.dt.float32)
                tb = pool_b.tile([_P, _S - CV], mybir.dt.float32)
                nc.sync.dma_start(ta[:], attn_in[rows, 0:CV])
                nc.sync.dma_start(tb[:], attn_in[rows, CV:_S])
                # cols [0, CV): elementwise accumulate across chunks on DVE.
                # The tiny scratch copy takes the chunk-DMA wait onto the DVE
                # instruction stream first, so the accumulate op below carries
                # only Tile's same-engine pipeline-hazard wait (walrus allows
                # a single sync wait per instruction).
                sc = cpool.tile([_P, 1], mybir.dt.float32, name=f"sc{i}", tag=f"sc{i}")
                nc.vector.tensor_copy(sc[:], ta[:, 0:1])
                if i == 0:
                    nc.vector.tensor_copy(acc_v[:], ta[:])
                else:
                    nc.vector.tensor_add(acc_v[:], acc_v[:], ta[:])
                # cols [CV, S): partition-reduce on PE, accumulating in PSUM
                for j in range(NB_PE):
                    nc.tensor.matmul(
                        accs[NB_V + j][:],
                        ones_t[:],
                        tb[:, _NBANK * j : _NBANK * (j + 1)],
                        start=(i == 0),
                        stop=(i == n_chunks - 1),
                    )

            # partition-reduce the DVE accumulator into the remaining banks
            for j in range(NB_V):
                nc.tensor.matmul(
                    accs[j][:],
                    ones_t[:],
                    acc_v[:, _NBANK * j : _NBANK * (j + 1)],
                    start=True,
                    stop=True,
                )

            res = rpool.tile([1, _S], mybir.dt.float32)
            for j in range(n_banks):
                nc.scalar.copy(res[0:1, _NBANK * j : _NBANK * (j + 1)], accs[j][:])
            nc.sync.dma_start(scores_out[:], res[:])

    # Walrus's direct2d DMA lowering accepts a single sync wait per DMACopy,
    # but Tile emits both the PE WAR wait (slot's readers done) and the
    # same-lane DMA WAW wait (slot's previous writer done) on buffer-reuse
    # DMAs. The DMA wait is transitively implied here: the matmuls that
    # incremented the PE semaphore each waited on that very DMA-lane value
    # before reading the slot. Drop it.
    insts = [i for blk in nc.m.functions[0].blocks for i in blk.instructions]
    last_dma = [i for i in insts if type(i).__name__ == "InstDMACopy"][-1]
    out_lane = last_dma.sync_info.on_update[0].ant_name

    for inst in insts:
        ty = type(inst).__name__
        si = inst.sync_info
        if si is None or len(si.on_wait) <= 1:
            continue
        if ty == "InstDMACopy":
            eng = [w for w in si.on_wait if not w.ant_name.startswith("DMAHW")]
            dma = [w for w in si.on_wait if w.ant_name.startswith("DMAHW")]
            assert len(eng) == 1 and len(eng) + len(dma) == len(si.on_wait), (
                f"unexpected DMA wait pattern: {si}"
            )
            inst.sync_info = mybir.SyncInfo(on_wait=eng, on_update=list(si.on_update))
        elif ty == "InstDrain":
            # Kernel-tail drain: every wait is transitively implied by the
            # completion of the final (scores) DMA, whose chain covers all
            # matmuls, copies and chunk DMAs. Keep only that lane's wait.
            keep = [w for w in si.on_wait if w.ant_name == out_lane]
            assert len(keep) == 1, f"unexpected drain wait pattern: {si}"
            inst.sync_info = mybir.SyncInfo(on_wait=keep, on_update=list(si.on_update))
    return nc


def _get_nc():
    global _cached_nc
    if _cached_nc is None:
        _cached_nc = _build_nc()
    return _cached_nc


# Set by test harnesses to capture a profile of the device run.
TRACE = False
LAST_EXEC_NS = None


_ldw_patch_done = False


def _enable_walrus_ldw_opt():
    """The 256 accumulating matmuls all reload the same 1-column ones vector;
    walrus's LDW dedup pass (disabled by default in concourse) removes the
    redundant loads so consecutive matmuls pipeline on the PE."""
    global _ldw_patch_done
    if _ldw_patch_done:
        return
    import concourse.bass_utils as bu

    orig = bu.run_command

    def patched(argv, **kw):
        argv = [
            a.replace("--enable-ldw-opt=false", "--enable-ldw-opt=true")
            if isinstance(a, str)
            else a
            for a in argv
        ]
        return orig(argv, **kw)

    bu.run_command = patched
    _ldw_patch_done = True


def _device_scores(attn):
    """attn: [8, 4096, 4096] f32 -> scores [8, 4096] f32 (device column sums)."""
    global LAST_EXEC_NS
    from concourse.bass_utils import run_bass_kernel_spmd

    _enable_walrus_ldw_opt()

    nc = _get_nc()
    in_maps = [{"attn": np.ascontiguousarray(attn[b])} for b in range(_B)]
    res = run_bass_kernel_spmd(nc, in_maps, list(range(_NCORES)), trace=TRACE)
    LAST_EXEC_NS = res.exec_time_ns
    return np.stack([np.asarray(res.results[b]["scores"]).reshape(_S) for b in range(_B)])


def _seq_colsum(mat, cols):
    """Exact reference-order (flat sequential over rows) f32 sums of selected
    columns of one [S, S] matrix. Bitwise-identical to jax CPU einsum."""
    sl = np.ascontiguousarray(mat[:, cols], dtype=np.float32)
    acc = sl[0].copy()
    for r in range(1, sl.shape[0]):
        acc = acc + sl[r]
    return acc


def _topk_sorted_exact(dev_scores, mat, k, delta):
    """Indices (ascending) of the k largest reference-order scores, using
    device scores for the bulk and exact host recomputation near the boundary.

    dev_scores: [S] device column sums; mat: [S, S] the batch matrix.
    """
    S = dev_scores.shape[0]
    v = np.partition(dev_scores, S - k)[S - k]  # k-th largest device score
    certain = np.nonzero(dev_scores > v + delta)[0]
    if certain.size > k:
        return None  # window too narrow for this data; caller widens/falls back
    cand = np.nonzero(
        (dev_scores >= v - delta) & (dev_scores <= v + delta)
    )[0]
    need = k - certain.size
    if need > cand.size:
        return None
    if need == 0:
        chosen = np.empty(0, dtype=np.int64)
    else:
        exact = _seq_colsum(mat, cand)
        # ties -> lower column index first (cand ascending + stable sort),
        # matching jax.lax.top_k
        order = np.argsort(-exact, kind="stable")
        chosen = cand[np.sort(order[:need])]
    return np.sort(np.concatenate([certain, chosen])).astype(np.int32)


def _reference_numpy(pad, attn):
    """Full-fidelity numpy fallback replicating the jax reference bit-exactly
    (flat sequential f32 accumulation). Used only off the fast path."""
    B, S = pad.shape
    k = max(1, int(S * _THROUGHPUT))
    lens = S - pad.sum(axis=1, dtype=np.float32)
    pos = np.arange(S, dtype=pad.dtype)
    valid = pos[None, :] < lens[:, None]
    vm = valid.astype(np.float32)
    acc = attn[:, 0, :] * vm[:, 0:1]
    for r in range(1, S):
        acc = acc + attn[:, r, :] * vm[:, r : r + 1]
    scores = np.where(valid, acc, -np.inf).astype(np.float32)
    x = np.empty((B, k), np.int32)
    for b in range(B):
        order = np.argsort(-scores[b], kind="stable")
        x[b] = np.sort(order[:k])
    return k, x


def kernel(pad, attn):
    pad = np.asarray(pad)
    attn = np.asarray(attn, dtype=np.float32)
    B, S = pad.shape
    k = max(1, int(S * _THROUGHPUT))

    if (B, S) == (_B, _S) and not np.any(pad):
        dev = _device_scores(attn)
        x = np.empty((B, k), np.int32)
        for b in range(B):
            got = None
            for delta in (_DELTA, 4 * _DELTA, 16 * _DELTA):
                got = _topk_sorted_exact(dev[b], attn[b], k, delta)
                if got is not None:
                    break
            if got is None:
                # device scores unusable for this batch: exact host fallback
                exact = _seq_colsum(attn[b], np.arange(S))
                order = np.argsort(-exact, kind="stable")
                got = np.sort(order[:k]).astype(np.int32)
            x[b] = got
    else:
        k, x = _reference_numpy(pad, attn)

    stride = np.int32(k)
    batch = np.repeat(np.arange(B, dtype=np.int32), k)
    y = np.tile(np.arange(k, dtype=np.int32), B)
    return stride, batch, x.reshape(-1), y
